# revision 4
# baseline (speedup 1.0000x reference)
"""NoauxTC MoE router (DeepSeek-style grouped top-k) on 8 Trainium2 cores.

Full-input contract: kernel(router_logits[131072,256] f32, e_score_correction_bias[256] f32)
-> (topk_weights[131072,8] f32, topk_ids[131072,8] int32)

Sharding: tokens split contiguously across 8 cores (data parallel), bias replicated.

Per-core layout: token t_local = p*128 + c  (p = SBUF partition, c = column 0..127).
Columns are processed in NB batches of KB tiles; each tile is [128 tokens, 256 experts].
"""

import sys

import numpy as np

if "/opt/trn_rl_repo" not in sys.path:
    sys.path.insert(0, "/opt/trn_rl_repo")

import concourse.bacc as bacc
import concourse.mybir as mybir
from concourse.bass_utils import run_bass_kernel_spmd
from concourse.tile import TileContext

N_CORES = 8
T_FULL = 131072
E = 256
N_GROUP = 8
PER_GROUP = E // N_GROUP  # 32
TOPK_GROUP = 4
TOP_K = 8
SCALE = 2.5

TLOC = T_FULL // N_CORES  # 16384
P = 128                   # partitions
NCOL = TLOC // P          # 128 tokens per partition
KB = 8                    # tiles (columns) per instruction batch
NB = NCOL // KB           # 16 batches

F32 = mybir.dt.float32
U32 = mybir.dt.uint32
AX = mybir.AxisListType
OP = mybir.AluOpType
ACT_SIGMOID = mybir.ActivationFunctionType.Sigmoid

NEG_BIG = -1.0e30


def build_program(tloc=TLOC):
    ncol = tloc // P
    nb = ncol // KB
    nc = bacc.Bacc("TRN2", target_bir_lowering=False)

    logits = nc.dram_tensor("logits", [tloc, E], F32, kind="ExternalInput")
    biasb = nc.dram_tensor("biasb", [P, E], F32, kind="ExternalInput")
    ids_o = nc.dram_tensor("ids", [tloc, TOP_K], U32, kind="ExternalOutput")
    vals_o = nc.dram_tensor("vals", [tloc, TOP_K], F32, kind="ExternalOutput")

    # token t = p*ncol + c ; view [P, ncol, *]
    lg_v = logits.rearrange("(p c) e -> p c e", p=P)
    id_v = ids_o.rearrange("(p c) j -> p c j", p=P)
    va_v = vals_o.rearrange("(p c) j -> p c j", p=P)

    with TileContext(nc) as tc:
        with tc.tile_pool(name="const", bufs=1) as cpool, \
             tc.tile_pool(name="work", bufs=2) as pool:
            bias_sb = cpool.tile([P, E], F32)
            nc.sync.dma_start(bias_sb[:], biasb[:])
            bias_bc = bias_sb[:].unsqueeze(1).to_broadcast([P, KB, E])

            for b in range(nb):
                csl = slice(b * KB, (b + 1) * KB)

                L = pool.tile([P, KB, E], F32)
                nc.sync.dma_start(L[:], lg_v[:, csl, :])

                # scores = sigmoid(logits)
                S = pool.tile([P, KB, E], F32)
                nc.scalar.activation(S[:], L[:], ACT_SIGMOID)

                # corrected = scores + bias
                C = pool.tile([P, KB, E], F32)
                nc.vector.tensor_add(C[:], S[:], bias_bc)

                Cg = C[:].rearrange("p k (g w) -> p (k g) w", w=PER_GROUP)

                # per-group top-2 via max / knockout / max
                m1 = pool.tile([P, KB * N_GROUP], F32)
                nc.vector.tensor_reduce(m1[:], Cg, axis=AX.X, op=OP.max)

                C2 = pool.tile([P, KB, E], F32)
                for k in range(KB):
                    nc.vector.match_replace(
                        out=C2[:, k, :],
                        in_to_replace=m1[:, k * N_GROUP:(k + 1) * N_GROUP],
                        in_values=C[:, k, :],
                        imm_value=NEG_BIG,
                    )
                C2g = C2[:].rearrange("p k (g w) -> p (k g) w", w=PER_GROUP)
                m2 = pool.tile([P, KB * N_GROUP], F32)
                nc.vector.tensor_reduce(m2[:], C2g, axis=AX.X, op=OP.max)

                gs = pool.tile([P, KB * N_GROUP], F32)
                nc.vector.tensor_add(gs[:], m1[:], m2[:])

                # rank of each group among its 8 (0 = best); selected iff rank <= 3
                gsr = gs[:].rearrange("p (k g) -> p k g", g=N_GROUP)
                in0 = gsr.unsqueeze(2).to_broadcast([P, KB, N_GROUP, N_GROUP])
                in1 = gsr.unsqueeze(3).to_broadcast([P, KB, N_GROUP, N_GROUP])
                R = pool.tile([P, KB, N_GROUP, N_GROUP], F32)
                nc.vector.tensor_tensor(R[:], in0, in1, OP.is_gt)
                rank = pool.tile([P, KB * N_GROUP], F32)
                nc.vector.tensor_reduce(rank[:], R[:], axis=AX.X, op=OP.add)

                # masked = (rank <= 3) * corrected   (one fused pass)
                rke = (rank[:].rearrange("p (k g) -> p k g", g=N_GROUP)
                       .unsqueeze(3).to_broadcast([P, KB, N_GROUP, PER_GROUP]))
                masked = pool.tile([P, KB, E], F32)
                nc.vector.scalar_tensor_tensor(
                    out=masked[:].rearrange("p k (g w) -> p k g w", w=PER_GROUP),
                    in0=rke,
                    scalar=float(TOPK_GROUP) - 0.5,
                    in1=C[:].rearrange("p k (g w) -> p k g w", w=PER_GROUP),
                    op0=OP.is_lt,
                    op1=OP.mult,
                )

                v8 = pool.tile([P, KB, TOP_K], F32)
                i8 = pool.tile([P, KB, TOP_K], U32)
                for k in range(KB):
                    nc.vector.max(out=v8[:, k, :], in_=masked[:, k, :])
                    nc.vector.max_index(
                        out=i8[:, k, :], in_max=v8[:, k, :], in_values=masked[:, k, :]
                    )

                nc.sync.dma_start(va_v[:, csl, :], v8[:])
                nc.sync.dma_start(id_v[:, csl, :], i8[:])

    nc.compile()
    return nc


_prog_cache = {}


def _get_program(tloc=TLOC):
    if tloc not in _prog_cache:
        _prog_cache[tloc] = build_program(tloc)
    return _prog_cache[tloc]


def kernel(router_logits, e_score_correction_bias):
    router_logits = np.ascontiguousarray(router_logits, dtype=np.float32)
    bias = np.ascontiguousarray(e_score_correction_bias, dtype=np.float32)
    assert router_logits.shape == (T_FULL, E)

    nc = _get_program()
    biasb = np.tile(bias[None, :], (P, 1))
    in_maps = [
        {"logits": router_logits[c * TLOC:(c + 1) * TLOC], "biasb": biasb}
        for c in range(N_CORES)
    ]
    res = run_bass_kernel_spmd(nc, in_maps, list(range(N_CORES))).results

    ids = np.concatenate([r["ids"] for r in res], axis=0)          # [T,8] uint32
    vals = np.concatenate([r["vals"] for r in res], axis=0)        # [T,8] f32 (corrected)

    ids_i = ids.astype(np.int32)
    # weights = uncorrected sigmoid scores at the selected experts, renormalized
    s = vals - bias[ids_i]
    w = s / (s.sum(axis=-1, keepdims=True) + 1e-20) * SCALE
    return w.astype(np.float32), ids_i
